# revision 1
# baseline (speedup 1.0000x reference)
"""Bass/Tile MHA kernel for trn2, sharded over 8 cores as (batch, head-group).

Each core handles one batch b and 3 heads. Inputs are host-prepared:
  qt, kt, vt : [D, S] bf16   — Q[b].T etc. (transposed + cast on host)
  mt         : [S, S] bf16   — mask[b,0].T  (mt[k, q] = mask[b,0,q,k]), 0/1
  wqt, wkt, wvt : [D, 3*DK] bf16 — W_X.T[:, head_cols]
  wot        : [3*DK, D] bf16    — W_O.T[head_rows, :]
Output:
  out : [S, D] bf16 — partial output (sum over the 4 head-groups of a batch
        gives the final output rows for that batch).

All data paths are bf16 (fp32 PSUM accumulation), which halves HBM traffic
vs fp32 and enables PE fast weight loads; the attention mask is resident in
SBUF, prefetched on the gpsimd DMA queue during the projection phase.
"""

import numpy as np

import concourse.bass as bass
import concourse.bacc as bacc
import concourse.tile as tile
import concourse.mybir as mybir

F32 = mybir.dt.float32
BF16 = mybir.dt.bfloat16
AF = mybir.ActivationFunctionType
ALU = mybir.AluOpType

D = 768
DK = 64
NH = 3          # heads per core
HD = NH * DK    # 192


def build_mha_nc(S=2048, n_cores=8, reps=1, hw_reps=1):
    ST = S // 128   # s-tiles (also attention k-tiles)
    QQ = S // 512   # q quarters
    KT6 = D // 128  # contraction tiles for projections

    nc = bacc.Bacc("TRN2", target_bir_lowering=False, debug=False,
                   num_devices=n_cores)

    qt_d = nc.dram_tensor("qt", [D, S], BF16, kind="ExternalInput")
    kt_d = nc.dram_tensor("kt", [D, S], BF16, kind="ExternalInput")
    vt_d = nc.dram_tensor("vt", [D, S], BF16, kind="ExternalInput")
    mt_d = nc.dram_tensor("mt", [S, S], BF16, kind="ExternalInput")
    wqt_d = nc.dram_tensor("wqt", [D, HD], BF16, kind="ExternalInput")
    wkt_d = nc.dram_tensor("wkt", [D, HD], BF16, kind="ExternalInput")
    wvt_d = nc.dram_tensor("wvt", [D, HD], BF16, kind="ExternalInput")
    wot_d = nc.dram_tensor("wot", [HD, D], BF16, kind="ExternalInput")
    out_d = nc.dram_tensor("out", [S, D], BF16, kind="ExternalOutput")
    scratch_d = nc.dram_tensor("recip_scratch", [NH, S], F32)

    import contextlib

    with tile.TileContext(nc) as tc:
      for _rep in range(reps):
       with (tc.For_i(0, hw_reps) if hw_reps > 1
             else contextlib.nullcontext()):
        with tc.tile_pool(name="perm", bufs=1) as perm:
          with (
            tc.tile_pool(name="weights", bufs=1) as wpool,
            tc.tile_pool(name="raw", bufs=2) as raw_pool,
            tc.tile_pool(name="psum_proj", bufs=3, space="PSUM") as psum_proj,
            tc.tile_pool(name="psum_v", bufs=2, space="PSUM") as psum_v_pool,
          ):
            # ---- persistent SBUF tensors ----
            qT_a = perm.tile([128, S], BF16, tag="qT_a")   # h0 rows 0-63, h1 64-127
            qT_b = perm.tile([64, S], BF16, tag="qT_b")    # h2
            kT_a = perm.tile([128, S], BF16, tag="kT_a")
            kT_b = perm.tile([64, S], BF16, tag="kT_b")
            v_sb = perm.tile([128, ST, NH, DK + 1], BF16, tag="v_sb")
            attnT_a = perm.tile([128, S], BF16, tag="attnT_a")
            attnT_b = perm.tile([64, S], BF16, tag="attnT_b")
            m_all = perm.tile([128, ST, S], BF16, tag="m_all")

            nc.vector.memset(v_sb[:], 1.0)

            # mask prefetch on the gpsimd queue: needed only in phase 2,
            # overlaps with the projection phase's sync-queue loads
            mt_t = mt_d.ap().rearrange("(t p) q -> p t q", p=128)
            for t in range(ST):
                nc.gpsimd.dma_start(m_all[:, t, :], mt_t[:, t, :])

            # ---- phase 1: projections ----
            wq_sb = wpool.tile([128, KT6, HD], BF16, tag="wq")
            wk_sb = wpool.tile([128, KT6, HD], BF16, tag="wk")
            wv_sb = wpool.tile([128, KT6, HD], BF16, tag="wv")
            nc.sync.dma_start(wq_sb[:], wqt_d.ap().rearrange("(o p) m -> p o m", p=128))
            nc.sync.dma_start(wk_sb[:], wkt_d.ap().rearrange("(o p) m -> p o m", p=128))
            nc.sync.dma_start(wv_sb[:], wvt_d.ap().rearrange("(o p) m -> p o m", p=128))

            def load_raw(x_dram, name):
                # per-kt DMAs: finer deps, more queue parallelism
                x_raw = raw_pool.tile([128, KT6, S], BF16, tag="raw", name=name)
                x_t = x_dram.ap().rearrange("(o p) s -> p o s", p=128)
                for kt in range(KT6):
                    nc.sync.dma_start(x_raw[:, kt, :], x_t[:, kt, :])
                return x_raw

            def project_T(x_raw, w_sb, dst_a, dst_b):
                # dst_a[0:128] = (x @ w[:, 0:128]).T ; dst_b[0:64] = (x @ w[:, 128:192]).T
                PW = min(1024, S)
                for mt_i, (dst, mw) in enumerate([(dst_a, 128), (dst_b, 64)]):
                    for w in range(S // PW):
                        ps = psum_proj.tile([128, PW], F32, tag="ps_proj",
                                            name="ps_proj")
                        for kt in range(KT6):
                            for half in range(PW // 512):
                                nc.tensor.matmul(
                                    ps[:mw, half * 512:(half + 1) * 512],
                                    w_sb[:, kt, mt_i * 128: mt_i * 128 + mw]
                                    ,
                                    x_raw[:, kt, w * PW + half * 512:
                                          w * PW + (half + 1) * 512],
                                    start=(kt == 0), stop=(kt == KT6 - 1))
                        nc.any.tensor_copy(
                            dst[:mw, w * PW:(w + 1) * PW], ps[:mw, :])

            q_raw = load_raw(qt_d, "q_raw")
            project_T(q_raw, wq_sb, qT_a, qT_b)
            k_raw = load_raw(kt_d, "k_raw")
            project_T(k_raw, wk_sb, kT_a, kT_b)

            # v projection: v[s, 3*64] in s-major layout (bf16 moving dim of
            # 192 runs at 1 cycle/row, no f32r small-moving penalty)
            v_raw = load_raw(vt_d, "v_raw")
            for st in range(ST):
                psv = psum_v_pool.tile([128, HD], F32, tag="psv")
                for kt in range(KT6):
                    nc.tensor.matmul(
                        psv[:],
                        v_raw[:, kt, st * 128:(st + 1) * 128],
                        wv_sb[:, kt, :],
                        start=(kt == 0), stop=(kt == KT6 - 1))
                nc.any.tensor_copy(
                    v_sb[:, st, :, 0:DK],
                    psv[:].rearrange("p (h d) -> p h d", h=NH))

          # ---- phase 2: attention (+ interleaved output projection) ----
          with (
            tc.tile_pool(name="expp", bufs=6) as exp_pool,
            tc.tile_pool(name="expm", bufs=6) as expm_pool,
            tc.tile_pool(name="ps_s", bufs=3, space="PSUM") as psum_s_pool,
            tc.tile_pool(name="ps_av", bufs=1, space="PSUM") as psum_av_pool,
            tc.tile_pool(name="norm", bufs=4) as norm_pool,
            tc.tile_pool(name="wo", bufs=1) as wo_pool,
            tc.tile_pool(name="outp", bufs=3) as out_pool,
            tc.tile_pool(name="ps_o", bufs=1, space="PSUM") as psum_o_pool,
          ):
            wot_a = wo_pool.tile([128, D], BF16, tag="wot_a")
            wot_b = wo_pool.tile([64, D], BF16, tag="wot_b")
            nc.sync.dma_start(wot_a[:], wot_d.ap()[0:128, :])
            nc.sync.dma_start(wot_b[:], wot_d.ap()[128:HD, :])

            head_src = [
                (qT_a, kT_a, 0),    # h0: partitions 0-63
                (qT_a, kT_a, 64),   # h1: partitions 64-127
                (qT_b, kT_b, 0),    # h2
            ]
            for qq in range(QQ):
                q0 = qq * 512
                avs = [psum_av_pool.tile([DK + 1, 512], F32, tag=f"av{h}",
                                         name=f"av{h}_{qq}")
                       for h in range(NH)]
                for kt in range(ST):
                    for h in range(NH):
                        qsrc, ksrc, p0 = head_src[h]
                        ps = psum_s_pool.tile([128, 512], F32, tag="s", name="ps_s")
                        nc.tensor.matmul(
                            ps[:],
                            ksrc[p0:p0 + DK, kt * 128:(kt + 1) * 128],
                            qsrc[p0:p0 + DK, q0:q0 + 512],
                            start=True, stop=True)
                        ex = exp_pool.tile([128, 512], BF16, tag="e", name="ex")
                        nc.scalar.activation(ex[:], ps[:], AF.Exp, scale=0.125)
                        em = expm_pool.tile([128, 512], BF16, tag="em", name="em")
                        nc.vector.tensor_tensor(
                            em[:], ex[:], m_all[:, kt, q0:q0 + 512], ALU.mult)
                        nc.tensor.matmul(
                            avs[h],
                            v_sb[:, kt, h, :],
                            em[:],
                            start=(kt == 0), stop=(kt == ST - 1))
                for h in range(NH):
                    rc = norm_pool.tile([1, 512], F32, tag="rc", name="rc")
                    nc.vector.reciprocal(rc[:], avs[h][DK:DK + 1, :])
                    bc = norm_pool.tile([64, 512], F32, tag="bc", name="bc")
                    nc.sync.dma_start(scratch_d.ap()[h, q0:q0 + 512], rc[:])
                    nc.sync.dma_start(
                        bc[:],
                        scratch_d.ap()[h, q0:q0 + 512].partition_broadcast(64))
                    if h < 2:
                        dst = attnT_a[h * 64:(h + 1) * 64, q0:q0 + 512]
                    else:
                        dst = attnT_b[0:64, q0:q0 + 512]
                    nc.vector.tensor_tensor(dst, avs[h][0:DK, :], bc[:], ALU.mult)

                # output projection for the 4 s-tiles covered by this qq block
                for st in range(qq * 4, qq * 4 + 4):
                    po = psum_o_pool.tile([128, D], F32, tag="po", name="po")
                    for (o, n) in [(0, 512), (512, 256)]:
                        nc.tensor.matmul(
                            po[:, o:o + n],
                            attnT_a[:, st * 128:(st + 1) * 128],
                            wot_a[:, o:o + n],
                            start=True, stop=False)
                        nc.tensor.matmul(
                            po[:, o:o + n],
                            attnT_b[0:64, st * 128:(st + 1) * 128],
                            wot_b[:, o:o + n],
                            start=False, stop=True)
                    ob = out_pool.tile([128, D], BF16, tag="ob", name="ob")
                    # pinned to DVE: ScalarE is saturated by the softmax exp
                    # stream in this phase, DVE has slack
                    nc.vector.tensor_copy(ob[:], po[:])
                    nc.sync.dma_start(out_d.ap()[st * 128:(st + 1) * 128, :], ob[:])

    nc.compile()
    return nc


def make_in_maps(Q, K, V, mask, W_Q, W_K, W_V, W_O, n_cores=8):
    import ml_dtypes
    bf16 = ml_dtypes.bfloat16
    in_maps = []
    for c in range(n_cores):
        b, g = divmod(c, 4)
        hs = slice(g * HD, (g + 1) * HD)
        in_maps.append({
            "qt": np.ascontiguousarray(Q[b].T).astype(bf16),
            "kt": np.ascontiguousarray(K[b].T).astype(bf16),
            "vt": np.ascontiguousarray(V[b].T).astype(bf16),
            "mt": np.ascontiguousarray(mask[b, 0].T).astype(bf16),
            "wqt": np.ascontiguousarray(W_Q.T[:, hs]).astype(bf16),
            "wkt": np.ascontiguousarray(W_K.T[:, hs]).astype(bf16),
            "wvt": np.ascontiguousarray(W_V.T[:, hs]).astype(bf16),
            "wot": np.ascontiguousarray(W_O.T[hs, :]).astype(bf16),
        })
    return in_maps


def combine_outputs(partials):
    partials = [np.asarray(p, np.float32) for p in partials]
    b0 = partials[0] + partials[1] + partials[2] + partials[3]
    b1 = partials[4] + partials[5] + partials[6] + partials[7]
    return np.stack([b0, b1])


_NC_CACHE = {}


def _get_nc(reps=1, hw_reps=1):
    key = ("nc", reps, hw_reps)
    if key not in _NC_CACHE:
        _NC_CACHE[key] = build_mha_nc(S=2048, n_cores=8,
                                      reps=reps, hw_reps=hw_reps)
    return _NC_CACHE[key]


def kernel(Q, K, V, mask, W_Q, W_K, W_V, W_O, _reps=1):
    from concourse.bass_utils import run_bass_kernel_spmd
    nc = _get_nc(_reps)
    in_maps = make_in_maps(np.asarray(Q, np.float32), np.asarray(K, np.float32),
                           np.asarray(V, np.float32), np.asarray(mask),
                           np.asarray(W_Q, np.float32), np.asarray(W_K, np.float32),
                           np.asarray(W_V, np.float32), np.asarray(W_O, np.float32))
    res = run_bass_kernel_spmd(nc, in_maps, core_ids=list(range(8)))
    out = combine_outputs([res.results[c]["out"] for c in range(8)])
    return out.astype(np.float32)



# revision 25
# speedup vs baseline: 1.0183x; 1.0183x over previous
"""Bass/Tile MHA kernel for trn2, sharded over 8 cores as (batch, head-group).

Each core handles one batch b and 3 heads. Inputs are host-prepared:
  qt, kt, vt : [D, S] bf16   — Q[b].T etc. (transposed + cast on host)
  mtk        : [128, ST, QQ, 512] bf16 — mask[b,0] per-k-tile:
               mtk[p, kt, qq, qi] = mask[b, 0, qq*512+qi, kt*128+p]
  wqt, wkt   : [D, 256] bf16 — W.T head cols packed [h0|h1|h2|h2]
               (h2 duplicated so qT_b/kT_b span all 128 partitions and the
                h2 scores can run as two concurrent row-group matmuls)
  wvt        : [D, 3*DK] bf16
  wot        : [3*DK, D] bf16    — W_O.T[head_rows, :]
Output:
  out : [S, D] bf16 — partial output (sum over the 4 head-groups of a batch
        gives the final output rows for that batch).

Schedule highlights:
  - all input DMAs ride one queue in consumption order (K first half, q
    quarter 0, then mask/V interleaved ...) so attention starts ~10us in;
    K's second half, V chunks and the remaining q quarters are projected
    inside the first attention sweep.
  - scores for h0 and h1 are issued as two matmuls on disjoint PE row
    groups (partitions 0-63 / 64-127) into the two banks of one
    [128,1024] PSUM tile — on HW they execute concurrently; one 1024-wide
    exp + one broadcast mask multiply covers both heads.  h2 gets the
    same shape by pairing adjacent k-tiles via its duplicated partitions.
  - softmax normalization reciprocal round-trips through DRAM on the
    (otherwise idle) gpsimd queue; output projections run one q-block
    behind between head sweeps.
"""

import numpy as np

import concourse.bass as bass
import concourse.bacc as bacc
import concourse.tile as tile
import concourse.mybir as mybir

F32 = mybir.dt.float32
BF16 = mybir.dt.bfloat16
AF = mybir.ActivationFunctionType
ALU = mybir.AluOpType

D = 768
DK = 64
NH = 3          # heads per core
HD = NH * DK    # 192


def build_mha_nc(S=2048, n_cores=8, reps=1, hw_reps=1):
    ST = S // 128   # k-tiles
    KP = ST // 2    # k-tile pairs
    QQ = S // 512   # q quarters
    KT6 = D // 128  # contraction tiles for projections

    nc = bacc.Bacc("TRN2", target_bir_lowering=False, debug=False,
                   num_devices=n_cores)

    qt_d = nc.dram_tensor("qt", [D, S], BF16, kind="ExternalInput")
    kt_d = nc.dram_tensor("kt", [D, S], BF16, kind="ExternalInput")
    vt_d = nc.dram_tensor("vt", [D, S], BF16, kind="ExternalInput")
    mtk_d = nc.dram_tensor("mtk", [128, ST, QQ, 512], BF16,
                           kind="ExternalInput")
    wqt_d = nc.dram_tensor("wqt", [D, 256], BF16, kind="ExternalInput")
    wkt_d = nc.dram_tensor("wkt", [D, 256], BF16, kind="ExternalInput")
    wvt_d = nc.dram_tensor("wvt", [D, HD], BF16, kind="ExternalInput")
    wot_d = nc.dram_tensor("wot", [HD, D], BF16, kind="ExternalInput")
    out_d = nc.dram_tensor("out", [S, D], BF16, kind="ExternalOutput")

    import contextlib

    with tile.TileContext(nc) as tc:
      for _rep in range(reps):
       with (tc.For_i(0, hw_reps) if hw_reps > 1
             else contextlib.nullcontext()):
        with (
            tc.tile_pool(name="perm", bufs=1) as perm,
            tc.tile_pool(name="weights", bufs=1) as wpool,
            tc.tile_pool(name="raw", bufs=1) as raw_pool,
            tc.tile_pool(name="ps", bufs=2, space="PSUM") as ps_pool,
            tc.tile_pool(name="av", bufs=2, space="PSUM") as av_pool,
            tc.tile_pool(name="po", bufs=2, space="PSUM") as po_pool,
            tc.tile_pool(name="expp", bufs=4) as exp_pool,
            tc.tile_pool(name="emp", bufs=4) as em_pool,
            tc.tile_pool(name="norm", bufs=3) as norm_pool,
            tc.tile_pool(name="outp", bufs=2) as out_pool,
        ):
            # ---- persistent SBUF tensors ----
            qT_a = perm.tile([128, S], BF16, tag="qT_a")   # h0 rows 0-63, h1 64-127
            qT_b = perm.tile([128, S], BF16, tag="qT_b")   # h2 duplicated
            kT_a = perm.tile([128, S], BF16, tag="kT_a")
            kT_b = perm.tile([128, S], BF16, tag="kT_b")
            v_sb = perm.tile([128, ST, NH, DK + 1], BF16, tag="v_sb")
            attnT_a = perm.tile([128, S], BF16, tag="attnT_a")
            attnT_b = perm.tile([64, S], BF16, tag="attnT_b")
            m_all = perm.tile([128, ST, QQ, 512], BF16, tag="m_all")
            ones_sb = perm.tile([1, 64], BF16, tag="ones")

            # preload the exp table set while phase-1 DMAs run
            dummy = norm_pool.tile([1, 8], F32, tag="dummy", name="dummy")
            nc.vector.memset(dummy[:], 0.0)
            nc.scalar.activation(dummy[:], dummy[:], AF.Exp, scale=1.0)

            nc.vector.memset(v_sb[:], 1.0)
            nc.vector.memset(ones_sb[:], 1.0)

            # ---- input DMAs in consumption order ----
            # masks ride the gpsimd queue (one fat DMA per k-tile, all four
            # q-blocks at once); everything else is on the hardware queue
            for kt in range(ST):
                nc.gpsimd.dma_start(m_all[:, kt], mtk_d.ap()[:, kt])

            wk_sb = wpool.tile([128, KT6, 256], BF16, tag="wk")
            nc.sync.dma_start(wk_sb[:], wkt_d.ap().rearrange("(o p) m -> p o m", p=128))
            wq_sb = wpool.tile([128, KT6, 256], BF16, tag="wq")
            nc.sync.dma_start(wq_sb[:], wqt_d.ap().rearrange("(o p) m -> p o m", p=128))
            wv_sb = wpool.tile([128, KT6, HD], BF16, tag="wv")
            nc.sync.dma_start(wv_sb[:], wvt_d.ap().rearrange("(o p) m -> p o m", p=128))

            k_raw = raw_pool.tile([128, KT6, S], BF16, tag="k_raw")
            k_t = kt_d.ap().rearrange("(o p) s -> p o s", p=128)
            nc.sync.dma_start(k_raw[:, :, 0:S // 2], k_t[:, :, 0:S // 2])

            q_raw = raw_pool.tile([128, KT6, S], BF16, tag="q_raw")
            q_t = qt_d.ap().rearrange("(o p) s -> p o s", p=128)
            nc.sync.dma_start(q_raw[:, :, 0:512], q_t[:, :, 0:512])

            v_raw = raw_pool.tile([128, KT6, S], BF16, tag="v_raw")
            v_t = vt_d.ap().rearrange("(o p) s -> p o s", p=128)

            def load_v(stp):
                # st-pair chunks keep the per-row DMA element >= 512B
                sc = slice(stp * 256, (stp + 1) * 256)
                nc.sync.dma_start(v_raw[:, :, sc], v_t[:, :, sc])

            for stp in range(4):
                load_v(stp)
            nc.sync.dma_start(k_raw[:, :, S // 2:S], k_t[:, :, S // 2:S])
            for stp in range(4, 8):
                load_v(stp)
            nc.sync.dma_start(q_raw[:, :, 512:1024], q_t[:, :, 512:1024])
            wot_a = wpool.tile([128, D], BF16, tag="wot_a")
            wot_b = wpool.tile([64, D], BF16, tag="wot_b")
            nc.sync.dma_start(wot_a[:], wot_d.ap()[0:128, :])
            nc.sync.dma_start(wot_b[:], wot_d.ap()[128:HD, :])
            nc.sync.dma_start(q_raw[:, :, 1024:1536], q_t[:, :, 1024:1536])
            nc.sync.dma_start(q_raw[:, :, 1536:2048], q_t[:, :, 1536:2048])

            # ---- projection helpers ----
            def project_k(mt_i, w):
                # kT_a (mt_i=0) / kT_b (mt_i=1), column window w*1024
                dst = kT_a if mt_i == 0 else kT_b
                ps = ps_pool.tile([128, 1024], F32, tag="ps", name="ps_k")
                for kt in range(KT6):
                    for half in range(2):
                        nc.tensor.matmul(
                            ps[:, half * 512:(half + 1) * 512],
                            wk_sb[:, kt, mt_i * 128:(mt_i + 1) * 128],
                            k_raw[:, kt, w * 1024 + half * 512:
                                  w * 1024 + (half + 1) * 512],
                            start=(kt == 0), stop=(kt == KT6 - 1))
                nc.vector.tensor_copy(
                    dst[:, w * 1024:(w + 1) * 1024], ps[:])

            def project_q_quarter(qq):
                q0 = qq * 512
                for mt_i, dst in enumerate([qT_a, qT_b]):
                    ps = po_pool.tile([128, 512], F32, tag="po",
                                      name=f"ps_q{mt_i}")
                    for kt in range(KT6):
                        nc.tensor.matmul(
                            ps[:], wq_sb[:, kt, mt_i * 128:(mt_i + 1) * 128],
                            q_raw[:, kt, q0:q0 + 512],
                            start=(kt == 0), stop=(kt == KT6 - 1))
                    nc.vector.tensor_copy(dst[:, q0:q0 + 512], ps[:])

            def project_v_st(st):
                psv = po_pool.tile([128, 512], F32, tag="po", name="psv")
                for kt in range(KT6):
                    nc.tensor.matmul(
                        psv[:, 0:HD],
                        v_raw[:, kt, st * 128:(st + 1) * 128],
                        wv_sb[:, kt, :],
                        start=(kt == 0), stop=(kt == KT6 - 1))
                nc.vector.tensor_copy(
                    v_sb[:, st, :, 0:DK],
                    psv[:, 0:HD].rearrange("p (h d) -> p h d", h=NH))

            project_k(0, 0)
            project_k(1, 0)
            project_q_quarter(0)
            project_v_st(0)
            project_v_st(1)
            next_v = [2]    # next V chunk to project (streamed into sweep 0)

            # ---- attention ----
            def out_proj(st):
                sc = slice(st * 128, (st + 1) * 128)
                ob = out_pool.tile([128, D], BF16, tag="ob", name="ob")
                for (o, n) in [(0, 512), (512, 256)]:
                    po = po_pool.tile([128, 512], F32, tag="po", name="po")
                    nc.tensor.matmul(po[:, 0:n], attnT_a[:, sc],
                                     wot_a[:, o:o + n],
                                     start=True, stop=False)
                    nc.tensor.matmul(po[:, 0:n], attnT_b[0:64, sc],
                                     wot_b[:, o:o + n],
                                     start=False, stop=True)
                    nc.vector.tensor_copy(ob[:, o:o + n], po[:, 0:n])
                nc.sync.dma_start(out_d.ap()[sc, :], ob[:])

            def normalize(av, h, q0):
                # 1/denominator, broadcast across partitions via a K=1
                # ones-matmul on the PE (no DRAM round-trip); the raw
                # attention values hop to SBUF so the final multiply reads
                # only one PSUM operand (hardware restriction)
                va = norm_pool.tile([64, 512], BF16, tag="va", name="va")
                nc.vector.tensor_copy(va[:], av[0:DK, :])
                rc = norm_pool.tile([1, 512], BF16, tag="rc", name="rc")
                with nc.allow_low_precision(reason="bf16 softmax denom recip"):
                    nc.vector.reciprocal(rc[:], av[DK:DK + 1, :])
                bc = po_pool.tile([128, 512], F32, tag="po", name="bc")
                nc.tensor.matmul(bc[0:64, :], ones_sb[:], rc[:],
                                 start=True, stop=True)
                if h < 2:
                    dst = attnT_a[h * 64:(h + 1) * 64, q0:q0 + 512]
                else:
                    dst = attnT_b[0:64, q0:q0 + 512]
                nc.vector.tensor_tensor(dst, va[:], bc[0:64, :], ALU.mult)

            for qq in range(QQ):
                q0 = qq * 512
                # --- h0 & h1 combined sweep (concurrent row groups) ---
                av0 = av_pool.tile([DK + 1, 512], F32, tag="av",
                                   name=f"av0_{qq}")
                av1 = av_pool.tile([DK + 1, 512], F32, tag="av",
                                   name=f"av1_{qq}")
                for kt in range(ST):
                    if qq == 0:
                        # stream in the rest of phase 1 behind the sweep
                        if kt == 5:
                            project_k(0, 1)
                        elif kt == 6:
                            project_k(1, 1)
                        else:
                            while next_v[0] <= min(kt + 2, ST - 1):
                                project_v_st(next_v[0])
                                next_v[0] += 1
                    ktc = slice(kt * 128, (kt + 1) * 128)
                    ps = ps_pool.tile([128, 1024], F32, tag="ps", name="ps_s")
                    nc.tensor.matmul(ps[:, 0:512],
                                     kT_a[0:DK, ktc], qT_a[0:DK, q0:q0 + 512],
                                     start=True, stop=True)
                    nc.tensor.matmul(ps[:, 512:1024],
                                     kT_a[DK:128, ktc], qT_a[DK:128, q0:q0 + 512],
                                     start=True, stop=True)
                    ex = exp_pool.tile([128, 1024], BF16, tag="e", name="ex")
                    nc.scalar.activation(ex[:], ps[:], AF.Exp, scale=0.125)
                    em = em_pool.tile([128, 1024], BF16, tag="em", name="em")
                    nc.vector.tensor_tensor(
                        em[:].rearrange("p (j q) -> p j q", j=2),
                        ex[:].rearrange("p (j q) -> p j q", j=2),
                        m_all[:, kt, qq].unsqueeze(1).broadcast_to([128, 2, 512]),
                        ALU.mult)
                    nc.tensor.matmul(av0[:], v_sb[:, kt, 0, :], em[:, 0:512],
                                     start=(kt == 0), stop=(kt == ST - 1))
                    nc.tensor.matmul(av1[:], v_sb[:, kt, 1, :], em[:, 512:1024],
                                     start=(kt == 0), stop=(kt == ST - 1))
                normalize(av0, 0, q0)
                normalize(av1, 1, q0)
                if qq + 1 < QQ:
                    project_q_quarter(qq + 1)
                if qq > 0:
                    out_proj((qq - 1) * 4 + 0)
                    out_proj((qq - 1) * 4 + 1)

                # --- h2 sweep (adjacent k-tiles on the two row groups) ---
                av2 = av_pool.tile([DK + 1, 512], F32, tag="av",
                                   name=f"av2_{qq}")
                for kp in range(KP):
                    k0c = slice((2 * kp) * 128, (2 * kp + 1) * 128)
                    k1c = slice((2 * kp + 1) * 128, (2 * kp + 2) * 128)
                    ps = ps_pool.tile([128, 1024], F32, tag="ps", name="ps_s2")
                    nc.tensor.matmul(ps[:, 0:512],
                                     kT_b[0:DK, k0c], qT_b[0:DK, q0:q0 + 512],
                                     start=True, stop=True)
                    nc.tensor.matmul(ps[:, 512:1024],
                                     kT_b[DK:128, k1c], qT_b[DK:128, q0:q0 + 512],
                                     start=True, stop=True)
                    ex = exp_pool.tile([128, 1024], BF16, tag="e", name="ex2")
                    nc.scalar.activation(ex[:], ps[:], AF.Exp, scale=0.125)
                    em = em_pool.tile([128, 1024], BF16, tag="em", name="em2")
                    nc.vector.tensor_tensor(
                        em[:].rearrange("p (j q) -> p j q", j=2),
                        ex[:].rearrange("p (j q) -> p j q", j=2),
                        m_all[:, 2 * kp:2 * kp + 2, qq, :],
                        ALU.mult)
                    nc.tensor.matmul(av2[:], v_sb[:, 2 * kp, 2, :],
                                     em[:, 0:512],
                                     start=(kp == 0), stop=False)
                    nc.tensor.matmul(av2[:], v_sb[:, 2 * kp + 1, 2, :],
                                     em[:, 512:1024],
                                     start=False, stop=(kp == KP - 1))
                normalize(av2, 2, q0)
                if qq > 0:
                    out_proj((qq - 1) * 4 + 2)
                    out_proj((qq - 1) * 4 + 3)

            for st in range((QQ - 1) * 4, QQ * 4):
                out_proj(st)

    nc.compile()
    return nc


def make_in_maps(Q, K, V, mask, W_Q, W_K, W_V, W_O, n_cores=8):
    import ml_dtypes
    bf16 = ml_dtypes.bfloat16
    S = Q.shape[1]
    ST, QQ = S // 128, S // 512
    in_maps = []
    for c in range(n_cores):
        b, g = divmod(c, 4)
        hs = slice(g * HD, (g + 1) * HD)
        mt = np.ascontiguousarray(mask[b, 0].T)      # [k, q]
        mk = mt.reshape(ST, 128, QQ, 512).transpose(1, 0, 2, 3)
        mk = np.ascontiguousarray(mk).astype(bf16)

        def dup_h2(w):
            wh = w.T[:, hs]                          # [768, 192]
            return np.ascontiguousarray(
                np.concatenate([wh, wh[:, 128:192]], axis=1)).astype(bf16)

        in_maps.append({
            "qt": np.ascontiguousarray(Q[b].T).astype(bf16),
            "kt": np.ascontiguousarray(K[b].T).astype(bf16),
            "vt": np.ascontiguousarray(V[b].T).astype(bf16),
            "mtk": mk,
            "wqt": dup_h2(W_Q),
            "wkt": dup_h2(W_K),
            "wvt": np.ascontiguousarray(W_V.T[:, hs]).astype(bf16),
            "wot": np.ascontiguousarray(W_O.T[hs, :]).astype(bf16),
        })
    return in_maps


def combine_outputs(partials):
    partials = [np.asarray(p, np.float32) for p in partials]
    b0 = partials[0] + partials[1] + partials[2] + partials[3]
    b1 = partials[4] + partials[5] + partials[6] + partials[7]
    return np.stack([b0, b1])


_NC_CACHE = {}


def _get_nc(reps=1, hw_reps=1):
    key = ("nc", reps, hw_reps)
    if key not in _NC_CACHE:
        _NC_CACHE[key] = build_mha_nc(S=2048, n_cores=8,
                                      reps=reps, hw_reps=hw_reps)
    return _NC_CACHE[key]


def kernel(Q, K, V, mask, W_Q, W_K, W_V, W_O, _reps=1):
    from concourse.bass_utils import run_bass_kernel_spmd
    nc = _get_nc(_reps)
    in_maps = make_in_maps(np.asarray(Q, np.float32), np.asarray(K, np.float32),
                           np.asarray(V, np.float32), np.asarray(mask),
                           np.asarray(W_Q, np.float32), np.asarray(W_K, np.float32),
                           np.asarray(W_V, np.float32), np.asarray(W_O, np.float32))
    res = run_bass_kernel_spmd(nc, in_maps, core_ids=list(range(8)))
    out = combine_outputs([res.results[c]["out"] for c in range(8)])
    return out.astype(np.float32)


# revision 39
# speedup vs baseline: 1.2526x; 1.2301x over previous
"""Bass/Tile MHA kernel for trn2, sharded over 8 cores as (batch, head-group).

Each core handles one batch b and 3 heads. Inputs are host-prepared:
  qt, kt, vt : [D, S] bf16   — Q[b].T etc. (transposed + cast on host)
  mtk        : [128, ST, QQ, 512] bf16 — mask[b,0] per-k-tile:
               mtk[p, kt, qq, qi] = mask[b, 0, qq*512+qi, kt*128+p]
  wqt, wkt   : [D, 256] bf16 — W.T head cols packed [h0|h1|h2|h2]
               (h2 duplicated so qT_b/kT_b span all 128 partitions and the
                h2 scores can run as two concurrent row-group matmuls)
  wvt        : [D, 3*DK] bf16
  wot        : [3*DK, D] bf16    — W_O.T[head_rows, :]
Output:
  out : [S, D] bf16 — partial output (sum over the 4 head-groups of a batch
        gives the final output rows for that batch).

Schedule highlights:
  - all input DMAs ride one queue in consumption order (K first half, q
    quarter 0, then mask/V interleaved ...) so attention starts ~10us in;
    K's second half, V chunks and the remaining q quarters are projected
    inside the first attention sweep.
  - scores for h0 and h1 are issued as two matmuls on disjoint PE row
    groups (partitions 0-63 / 64-127) into the two banks of one
    [128,1024] PSUM tile — on HW they execute concurrently; one 1024-wide
    exp + one broadcast mask multiply covers both heads.  h2 gets the
    same shape by pairing adjacent k-tiles via its duplicated partitions.
  - softmax normalization reciprocal round-trips through DRAM on the
    (otherwise idle) gpsimd queue; output projections run one q-block
    behind between head sweeps.
"""

import numpy as np

import concourse.bass as bass
import concourse.bacc as bacc
import concourse.tile as tile
import concourse.mybir as mybir

F32 = mybir.dt.float32
BF16 = mybir.dt.bfloat16
AF = mybir.ActivationFunctionType
ALU = mybir.AluOpType

D = 768
DK = 64
NH = 3          # heads per core
HD = NH * DK    # 192


def build_mha_nc(S=2048, n_cores=8, reps=1, hw_reps=1, attn_only=False):
    ST = S // 128   # k-tiles
    KP = ST // 2    # k-tile pairs
    QQ = S // 512   # q quarters
    KT6 = D // 128  # contraction tiles for projections

    nc = bacc.Bacc("TRN2", target_bir_lowering=False, debug=False,
                   num_devices=n_cores)

    qt_d = nc.dram_tensor("qt", [D, S], BF16, kind="ExternalInput")
    kt_d = nc.dram_tensor("kt", [D, S], BF16, kind="ExternalInput")
    vt_d = nc.dram_tensor("vt", [D, S], BF16, kind="ExternalInput")
    mtk_d = nc.dram_tensor("mtk", [128, ST, QQ, 512], BF16,
                           kind="ExternalInput")
    wqt_d = nc.dram_tensor("wqt", [D, 256], BF16, kind="ExternalInput")
    wkt_d = nc.dram_tensor("wkt", [D, 256], BF16, kind="ExternalInput")
    wvt_d = nc.dram_tensor("wvt", [D, HD], BF16, kind="ExternalInput")
    wot_d = nc.dram_tensor("wot", [HD, D], BF16, kind="ExternalInput")
    out_d = nc.dram_tensor("out", [S, D], BF16, kind="ExternalOutput")

    import contextlib

    with tile.TileContext(nc) as tc:
      for _rep in range(reps):
       with (tc.For_i(0, hw_reps) if (hw_reps > 1 and not attn_only)
             else contextlib.nullcontext()):
        with (
            tc.tile_pool(name="perm", bufs=1) as perm,
            tc.tile_pool(name="weights", bufs=1) as wpool,
            tc.tile_pool(name="raw", bufs=1) as raw_pool,
            tc.tile_pool(name="ps", bufs=2, space="PSUM") as ps_pool,
            tc.tile_pool(name="av", bufs=2, space="PSUM") as av_pool,
            tc.tile_pool(name="po", bufs=2, space="PSUM") as po_pool,
            tc.tile_pool(name="expp", bufs=5) as exp_pool,
            tc.tile_pool(name="emp", bufs=5) as em_pool,
            tc.tile_pool(name="norm", bufs=3) as norm_pool,
            tc.tile_pool(name="outp", bufs=2) as out_pool,
        ):
            # ---- persistent SBUF tensors ----
            qT_a = perm.tile([128, S], BF16, tag="qT_a")   # h0 rows 0-63, h1 64-127
            qT_b = perm.tile([128, S], BF16, tag="qT_b")   # h2 duplicated
            kT_a = perm.tile([128, S], BF16, tag="kT_a")
            kT_b = perm.tile([128, S], BF16, tag="kT_b")
            v_sb = perm.tile([128, ST, NH, DK + 1], BF16, tag="v_sb")
            attnT_a = perm.tile([128, S], BF16, tag="attnT_a")
            attnT_b = perm.tile([64, S], BF16, tag="attnT_b")
            m_all = perm.tile([128, ST, QQ, 512], BF16, tag="m_all")
            ones_sb = perm.tile([1, 64], BF16, tag="ones")

            # preload the exp table set while phase-1 DMAs run
            dummy = norm_pool.tile([1, 8], F32, tag="dummy", name="dummy")
            nc.vector.memset(dummy[:], 0.0)
            nc.scalar.activation(dummy[:], dummy[:], AF.Exp, scale=1.0)

            nc.vector.memset(v_sb[:, :, :, DK:DK + 1], 1.0)
            nc.vector.memset(ones_sb[:], 1.0)

            # ---- input DMAs on the sync queue, strictly in sweep-0
            # consumption order (each For_i iteration cold-starts behind an
            # all-engine barrier, so prefetch across iterations is moot) ----
            wk_sb = wpool.tile([128, KT6, 256], BF16, tag="wk")
            nc.sync.dma_start(wk_sb[:], wkt_d.ap().rearrange("(o p) m -> p o m", p=128))

            k_raw = raw_pool.tile([128, KT6, S], BF16, tag="k_raw")
            k_t = kt_d.ap().rearrange("(o p) s -> p o s", p=128)
            nc.sync.dma_start(k_raw[:, :, 0:512], k_t[:, :, 0:512])

            wq_sb = wpool.tile([128, KT6, 256], BF16, tag="wq")
            nc.sync.dma_start(wq_sb[:], wqt_d.ap().rearrange("(o p) m -> p o m", p=128))
            q_raw = raw_pool.tile([128, KT6, S], BF16, tag="q_raw")
            q_t = qt_d.ap().rearrange("(o p) s -> p o s", p=128)
            nc.sync.dma_start(q_raw[:, :, 0:512], q_t[:, :, 0:512])
            nc.sync.dma_start(k_raw[:, :, 512:1024], k_t[:, :, 512:1024])

            wv_sb = wpool.tile([128, KT6, HD], BF16, tag="wv")
            nc.sync.dma_start(wv_sb[:], wvt_d.ap().rearrange("(o p) m -> p o m", p=128))

            v_raw = raw_pool.tile([128, KT6, S], BF16, tag="v_raw")
            v_t = vt_d.ap().rearrange("(o p) s -> p o s", p=128)

            def load_v(stp):
                # st-pair chunks keep the per-row DMA element >= 512B
                sc = slice(stp * 256, (stp + 1) * 256)
                nc.sync.dma_start(v_raw[:, :, sc], v_t[:, :, sc])

            def load_m(kt):
                # sweep-0 slice only (2MB total in the critical window)
                nc.sync.dma_start(m_all[:, kt, 0], mtk_d.ap()[:, kt, 0])

            # consumption order: every For_i iteration cold-starts (the loop
            # reset is an all-engine barrier), so mask tiles interleave with
            # V chunks at the rate sweep 0 consumes them
            load_m(0); load_m(1)
            load_v(0)
            load_m(2); load_m(3)
            load_v(1)
            nc.sync.dma_start(k_raw[:, :, 1024:1536], k_t[:, :, 1024:1536])
            load_m(4); load_m(5)
            load_v(2)
            nc.sync.dma_start(k_raw[:, :, 1536:2048], k_t[:, :, 1536:2048])
            load_m(6); load_m(7)
            load_v(3)
            load_m(8); load_m(9)
            load_v(4)
            load_m(10); load_m(11)
            load_v(5)
            load_m(12); load_m(13)
            load_v(6)
            load_m(14); load_m(15)
            load_v(7)
            nc.sync.dma_start(q_raw[:, :, 512:1024], q_t[:, :, 512:1024])
            nc.sync.dma_start(m_all[:, :, 1, :], mtk_d.ap()[:, :, 1, :])
            wot_a = wpool.tile([128, D], BF16, tag="wot_a")
            wot_b = wpool.tile([64, D], BF16, tag="wot_b")
            nc.sync.dma_start(wot_a[:], wot_d.ap()[0:128, :])
            nc.sync.dma_start(wot_b[:], wot_d.ap()[128:HD, :])
            nc.sync.dma_start(q_raw[:, :, 1024:1536], q_t[:, :, 1024:1536])
            nc.sync.dma_start(m_all[:, :, 2, :], mtk_d.ap()[:, :, 2, :])
            nc.sync.dma_start(q_raw[:, :, 1536:2048], q_t[:, :, 1536:2048])
            nc.sync.dma_start(m_all[:, :, 3, :], mtk_d.ap()[:, :, 3, :])

            # ---- projection helpers ----
            def project_k_chunk(mt_i, c):
                # kT_a (mt_i=0) / kT_b (mt_i=1), 512-col window c
                dst = kT_a if mt_i == 0 else kT_b
                k0c = c * 512
                ps = po_pool.tile([128, 512], F32, tag="po", name="ps_k")
                for kt in range(KT6):
                    nc.tensor.matmul(
                        ps[:],
                        wk_sb[:, kt, mt_i * 128:(mt_i + 1) * 128],
                        k_raw[:, kt, k0c:k0c + 512],
                        start=(kt == 0), stop=(kt == KT6 - 1))
                nc.vector.tensor_copy(dst[:, k0c:k0c + 512], ps[:])

            def project_q_quarter(qq):
                q0 = qq * 512
                for mt_i, dst in enumerate([qT_a, qT_b]):
                    ps = po_pool.tile([128, 512], F32, tag="po",
                                      name=f"ps_q{mt_i}")
                    for kt in range(KT6):
                        nc.tensor.matmul(
                            ps[:], wq_sb[:, kt, mt_i * 128:(mt_i + 1) * 128],
                            q_raw[:, kt, q0:q0 + 512],
                            start=(kt == 0), stop=(kt == KT6 - 1))
                    nc.vector.tensor_copy(dst[:, q0:q0 + 512], ps[:])

            def project_v_st(st):
                psv = po_pool.tile([128, 512], F32, tag="po", name="psv")
                for kt in range(KT6):
                    nc.tensor.matmul(
                        psv[:, 0:HD],
                        v_raw[:, kt, st * 128:(st + 1) * 128],
                        wv_sb[:, kt, :],
                        start=(kt == 0), stop=(kt == KT6 - 1))
                nc.vector.tensor_copy(
                    v_sb[:, st, :, 0:DK],
                    psv[:, 0:HD].rearrange("p (h d) -> p h d", h=NH))

            project_k_chunk(0, 0)
            project_k_chunk(0, 1)
            project_q_quarter(0)
            project_v_st(0)
            project_v_st(1)
            next_v = [2]    # next V chunk to project (streamed into sweep 0)
            if attn_only:
                for c_ in range(4):
                    project_k_chunk(1, c_)
                project_k_chunk(0, 2)
                project_k_chunk(0, 3)
                for qq_ in range(1, QQ):
                    project_q_quarter(qq_)
                for st_ in range(2, ST):
                    project_v_st(st_)

            # ---- attention ----
            work = []       # deferred PE chunks, drained inside sweeps

            def _op_chunk(st, o, n, ob_box):
                def run():
                    sc = slice(st * 128, (st + 1) * 128)
                    if ob_box[0] is None:
                        ob_box[0] = out_pool.tile([128, D], BF16, tag="ob",
                                                  name="ob")
                    ob = ob_box[0]
                    po = po_pool.tile([128, 512], F32, tag="po", name="po")
                    nc.tensor.matmul(po[:, 0:n], attnT_a[:, sc],
                                     wot_a[:, o:o + n],
                                     start=True, stop=False)
                    nc.tensor.matmul(po[:, 0:n], attnT_b[0:64, sc],
                                     wot_b[:, o:o + n],
                                     start=False, stop=True)
                    nc.vector.tensor_copy(ob[:, o:o + n], po[:, 0:n])
                    if o + n == D:
                        nc.gpsimd.dma_start(out_d.ap()[sc, :], ob[:])
                return run

            def queue_q_proj(qq):
                def chunk(mt_i, dst):
                    def run():
                        q0 = qq * 512
                        ps = po_pool.tile([128, 512], F32, tag="po",
                                          name=f"ps_q{mt_i}")
                        for kt in range(KT6):
                            nc.tensor.matmul(
                                ps[:],
                                wq_sb[:, kt, mt_i * 128:(mt_i + 1) * 128],
                                q_raw[:, kt, q0:q0 + 512],
                                start=(kt == 0), stop=(kt == KT6 - 1))
                        nc.vector.tensor_copy(dst[:, q0:q0 + 512], ps[:])
                    return run
                work.append(chunk(0, qT_a))
                work.append(chunk(1, qT_b))

            def queue_out_proj(st):
                ob_box = [None]
                work.append(_op_chunk(st, 0, 512, ob_box))
                work.append(_op_chunk(st, 512, 256, ob_box))

            def drain_one():
                if work:
                    work.pop(0)()

            def normalize(av, h, q0):
                # 1/denominator broadcast across partitions via a K=1
                # ones-matmul on the PE, landing in the av tile's unused
                # partitions 64:128; raw attention values hop to SBUF so the
                # final multiply reads only one PSUM operand (hw restriction)
                va = norm_pool.tile([64, 512], BF16, tag="va", name="va")
                nc.vector.tensor_copy(va[:], av[0:DK, :])
                rc = norm_pool.tile([1, 512], BF16, tag="rc", name="rc")
                with nc.allow_low_precision(reason="bf16 softmax denom recip"):
                    nc.vector.reciprocal(rc[:], av[DK:DK + 1, :])
                nc.tensor.matmul(av[DK:128, :], ones_sb[:], rc[:],
                                 start=True, stop=True)
                if h < 2:
                    dst = attnT_a[h * 64:(h + 1) * 64, q0:q0 + 512]
                else:
                    dst = attnT_b[0:64, q0:q0 + 512]
                nc.vector.tensor_tensor(dst, va[:], av[DK:128, :], ALU.mult)

            if not attn_only:
                # remaining K projections drain inside sweep 0: kT_b's lower
                # half before the h2 sweep, upper halves before k-tiles 8-15
                def _kc(mt_i, c):
                    return lambda: project_k_chunk(mt_i, c)
                work.extend([_kc(1, 0), _kc(1, 1), _kc(0, 2), _kc(0, 3),
                             _kc(1, 2), _kc(1, 3)])

            attn_loop = (tc.For_i(0, hw_reps) if (hw_reps > 1 and attn_only)
                         else contextlib.nullcontext())
            with attn_loop:
             for qq in range(QQ):
                q0 = qq * 512
                # --- h0 & h1 combined sweep (concurrent row groups) ---
                av0 = av_pool.tile([128, 512], F32, tag="av",
                                   name=f"av0_{qq}")
                av1 = av_pool.tile([128, 512], F32, tag="av",
                                   name=f"av1_{qq}")
                for kt in range(ST):
                    if qq == 0 and not attn_only:
                        # stream in the rest of phase 1 behind the sweep
                        while next_v[0] <= min(kt + 2, ST - 1):
                            project_v_st(next_v[0])
                            next_v[0] += 1
                    ktc = slice(kt * 128, (kt + 1) * 128)
                    ps = ps_pool.tile([128, 1024], F32, tag="ps", name="ps_s")
                    nc.tensor.matmul(ps[:, 0:512],
                                     kT_a[0:DK, ktc], qT_a[0:DK, q0:q0 + 512],
                                     start=True, stop=True)
                    nc.tensor.matmul(ps[:, 512:1024],
                                     kT_a[DK:128, ktc], qT_a[DK:128, q0:q0 + 512],
                                     start=True, stop=True)
                    ex = exp_pool.tile([128, 1024], BF16, tag="e", name="ex")
                    nc.scalar.activation(ex[:], ps[:], AF.Exp, scale=0.125)
                    em = em_pool.tile([128, 1024], BF16, tag="em", name="em")
                    nc.vector.tensor_tensor(
                        em[:].rearrange("p (j q) -> p j q", j=2),
                        ex[:].rearrange("p (j q) -> p j q", j=2),
                        m_all[:, kt, qq].unsqueeze(1).broadcast_to([128, 2, 512]),
                        ALU.mult)
                    nc.tensor.matmul(av0[0:DK + 1, :], v_sb[:, kt, 0, :],
                                     em[:, 0:512],
                                     start=(kt == 0), stop=(kt == ST - 1))
                    nc.tensor.matmul(av1[0:DK + 1, :], v_sb[:, kt, 1, :],
                                     em[:, 512:1024],
                                     start=(kt == 0), stop=(kt == ST - 1))
                    if kt % 2 == 1:
                        drain_one()
                normalize(av0, 0, q0)
                normalize(av1, 1, q0)
                if qq + 1 < QQ and not attn_only:
                    queue_q_proj(qq + 1)

                # --- h2 sweep (adjacent k-tiles on the two row groups) ---
                av2 = av_pool.tile([128, 512], F32, tag="av",
                                   name=f"av2_{qq}")
                for kp in range(KP):
                    k0c = slice((2 * kp) * 128, (2 * kp + 1) * 128)
                    k1c = slice((2 * kp + 1) * 128, (2 * kp + 2) * 128)
                    ps = ps_pool.tile([128, 1024], F32, tag="ps", name="ps_s2")
                    nc.tensor.matmul(ps[:, 0:512],
                                     kT_b[0:DK, k0c], qT_b[0:DK, q0:q0 + 512],
                                     start=True, stop=True)
                    nc.tensor.matmul(ps[:, 512:1024],
                                     kT_b[DK:128, k1c], qT_b[DK:128, q0:q0 + 512],
                                     start=True, stop=True)
                    ex = exp_pool.tile([128, 1024], BF16, tag="e", name="ex2")
                    nc.scalar.activation(ex[:], ps[:], AF.Exp, scale=0.125)
                    em = em_pool.tile([128, 1024], BF16, tag="em", name="em2")
                    nc.vector.tensor_tensor(
                        em[:].rearrange("p (j q) -> p j q", j=2),
                        ex[:].rearrange("p (j q) -> p j q", j=2),
                        m_all[:, 2 * kp:2 * kp + 2, qq, :],
                        ALU.mult)
                    nc.tensor.matmul(av2[0:DK + 1, :], v_sb[:, 2 * kp, 2, :],
                                     em[:, 0:512],
                                     start=(kp == 0), stop=False)
                    nc.tensor.matmul(av2[0:DK + 1, :], v_sb[:, 2 * kp + 1, 2, :],
                                     em[:, 512:1024],
                                     start=False, stop=(kp == KP - 1))
                    if kp % 2 == 1:
                        drain_one()
                normalize(av2, 2, q0)
                for st in range(qq * 4, qq * 4 + 4):
                    queue_out_proj(st)

             while work:
                drain_one()

    nc.compile()
    return nc


def make_in_maps(Q, K, V, mask, W_Q, W_K, W_V, W_O, n_cores=8):
    import ml_dtypes
    bf16 = ml_dtypes.bfloat16
    S = Q.shape[1]
    ST, QQ = S // 128, S // 512
    in_maps = []
    for c in range(n_cores):
        b, g = divmod(c, 4)
        hs = slice(g * HD, (g + 1) * HD)
        mt = np.ascontiguousarray(mask[b, 0].T)      # [k, q]
        mk = mt.reshape(ST, 128, QQ, 512).transpose(1, 0, 2, 3)
        mk = np.ascontiguousarray(mk).astype(bf16)

        def dup_h2(w):
            wh = w.T[:, hs]                          # [768, 192]
            return np.ascontiguousarray(
                np.concatenate([wh, wh[:, 128:192]], axis=1)).astype(bf16)

        in_maps.append({
            "qt": np.ascontiguousarray(Q[b].T).astype(bf16),
            "kt": np.ascontiguousarray(K[b].T).astype(bf16),
            "vt": np.ascontiguousarray(V[b].T).astype(bf16),
            "mtk": mk,
            "wqt": dup_h2(W_Q),
            "wkt": dup_h2(W_K),
            "wvt": np.ascontiguousarray(W_V.T[:, hs]).astype(bf16),
            "wot": np.ascontiguousarray(W_O.T[hs, :]).astype(bf16),
        })
    return in_maps


def combine_outputs(partials):
    partials = [np.asarray(p, np.float32) for p in partials]
    b0 = partials[0] + partials[1] + partials[2] + partials[3]
    b1 = partials[4] + partials[5] + partials[6] + partials[7]
    return np.stack([b0, b1])


_NC_CACHE = {}


def _get_nc(reps=1, hw_reps=1):
    key = ("nc", reps, hw_reps)
    if key not in _NC_CACHE:
        _NC_CACHE[key] = build_mha_nc(S=2048, n_cores=8,
                                      reps=reps, hw_reps=hw_reps)
    return _NC_CACHE[key]


def kernel(Q, K, V, mask, W_Q, W_K, W_V, W_O, _reps=1):
    from concourse.bass_utils import run_bass_kernel_spmd
    nc = _get_nc(_reps)
    in_maps = make_in_maps(np.asarray(Q, np.float32), np.asarray(K, np.float32),
                           np.asarray(V, np.float32), np.asarray(mask),
                           np.asarray(W_Q, np.float32), np.asarray(W_K, np.float32),
                           np.asarray(W_V, np.float32), np.asarray(W_O, np.float32))
    res = run_bass_kernel_spmd(nc, in_maps, core_ids=list(range(8)))
    out = combine_outputs([res.results[c]["out"] for c in range(8)])
    return out.astype(np.float32)
